# revision 30
# baseline (speedup 1.0000x reference)
"""CliffordLinearSimple on 8 Trainium2 NeuronCores.

Math (per reference):
    sv   = x[:, :, SV_IDX]                      # [B, IN_F, 9]  (scalar+vector slots)
    svo  = sv.reshape(B, IN_F*9) @ W.T + b      # [B, OUT_F*9]
    v    = svo.reshape(B, OUT_F, 9)[:, :, 1:]   # [B, OUT_F, 8]
    biv  = v[:, :, IU] * v[:, :, JU]            # [B, OUT_F, 28]
    out[..., SV_IDX] = svo; out[..., BIV_IDX] = biv; rest 0

Distribution: tensor-parallel over OUT_F (row-split W): core c owns out
features [c*128, (c+1)*128).  Every core gets the full sv (gathered and
transposed on host -- only 9/256 of x's last dim is ever read), and its
W row shard pre-packed to the PE's [K, N] layout in bf16.

The device does ONLY the GEMM (bf16 operands, fp32 PSUM) and writes the
[256, 1152] result back as bf16 (~0.6MB): bias add, the 28 bivector
products, and the scatter into the [256, 1024, 256] multivector output
happen on the host in fp32.

Schedule (v2, diagonal k-outer): the N=1152 columns are three PSUM
tiles (432, 504, 216); K=9216 is 72 k-tiles in groups
G=(6,9,9,12,12,12,12).  Step t runs W-blocks (g=t, n=0), (g=t-1, n=1),
(g=t-2, n=2) -- a diagonal wavefront.  This (a) streams W and svT at a
uniform byte rate matched to PE consumption with no phase cliffs (the
old n-outer schedule front-loaded svT + the first column tile and
stalled the PE ~4.4us at phase boundaries), (b) opens with a split
2-ktile W chunk so real matmuls begin ~1.5us after the framework
preamble, and (c) finishes the three tiles staggered (steps 6, 7, 8),
so mid drains overlap compute on SWDGE and only the narrow 216-wide
tile's drain is in the serial tail, split across the then-idle HWDGE
rings.  A handful of K=1 junk warm-up matmuls (HAM-invisible by
design; see the junk() comment) keep the PE queue alive between the
preamble and the first W chunk.
"""
import sys

if "/opt/trn_rl_repo" not in sys.path:
    sys.path.insert(0, "/opt/trn_rl_repo")

from contextlib import ExitStack

import ml_dtypes
import numpy as np

import concourse.bass as bass
import concourse.tile as tile
from concourse import bacc, mybir
from concourse.bass_utils import run_bass_kernel_spmd

ALG_DIM = 8
D1 = 9
MV_DIM = 256
B, IN_F, OUT_F = 256, 1024, 1024
POW2 = np.array([2 ** i for i in range(ALG_DIM)])
SV_IDX = np.concatenate([[0], POW2])
IU, JU = np.triu_indices(ALG_DIM, 1)
BIV_IDX = POW2[IU] + POW2[JU]
NCORES = 8
OF = OUT_F // NCORES  # 128 out features per core
N_CORE = OF * D1      # 1152 out slots per core

# K = IN_F*9 = 9216 = KT*128.  KGRP: k-group sizes (in 128-deep k-tiles)
# shared by all three column tiles; NTILES: PSUM tile widths (<=512
# f32/bank), narrowest last so the final drain is smallest.  Groups are
# big (>=6kt) so each W block carries >=2us of PE work -- with small
# ramped groups the ~0.65us/DMA ring-issue latency plus transfer time
# couldn't hide inside the tiny early steps and the PE chased the DMA
# stream just-in-time-late for the whole first half (measured +13.6us
# of ramp stalls).  Only the very first (g=0, n=0) block is split 2+4
# ktiles so the opening matmuls start ~1us earlier.
KGRP = (6, 9, 9, 12, 12, 12, 12)
FULL_CFG = dict(KT=72, KTLS=(KGRP, KGRP, KGRP), OF=128, NTILES=(432, 504, 216), BT=2, WARM=16)

# ramp-phase DMA splits (in ktiles) for W block (0,0) and svT chunk 0 --
# the opening critical path only.  Finer splits were tried and HURT:
# sub-2KB partition lines crater SDMA descriptor efficiency and each
# piece pays a ~0.7us fixed floor.
W_SPLITS = {(0, 0): (2, 4)}
SVT_SPLITS = {0: (2, 4)}

# junk-matmul padding (count of junk MMs) emitted after the i-th
# (0-based) ramp seam, keyed by (step t, index in that step's MM
# emission).  Empty: static pads execute when the PE reaches them, not
# when a stall actually happens, so they cost their full duration on a
# fast device -- measured as pure overhead (+3.6us).
PADS = {}


def build_core_program(KT, KTLS, OF, NTILES, BT, WARM=0):
    """SPMD per-core program: C[128*BT, OF*9] = svT.T @ Wh, written back as
    bf16 (bias + bivector products happen on the host)."""
    assert all(KT == sum(k) for k in KTLS) and sum(NTILES) == OF * D1
    NT = len(NTILES)
    assert len(KTLS) == NT
    KGRPS = KTLS[0]
    assert all(tuple(k) == tuple(KGRPS) for k in KTLS)
    NG = len(KGRPS)
    NOFF = [sum(NTILES[:i]) for i in range(NT)]          # column offsets
    KOFF = [sum(KGRPS[:i]) for i in range(NG)]           # k-group offsets
    Bfull = BT * 128
    f32, bf16 = mybir.dt.float32, mybir.dt.bfloat16

    nc = bacc.Bacc("TRN2", target_bir_lowering=False, debug=False)
    svT_d = nc.dram_tensor("svT", [128, KT, Bfull], bf16, kind="ExternalInput").ap()
    # flat per-n W: k-group blocks [128, ktl, NTILE] packed contiguously in
    # group order, so every DMA reads one fully-sequential DRAM region
    W_ds = [
        nc.dram_tensor(f"Wh{n}", [KT * 128 * NTILES[n]], bf16, kind="ExternalInput").ap()
        for n in range(NT)
    ]
    # [p, m*1152 + j] = C[m*128 + p, j]: per-partition output lines are
    # contiguous, so each drain is one 128-descriptor DMA
    out_d = nc.dram_tensor("outc", [128, BT * OF * D1], bf16, kind="ExternalOutput").ap()

    rings = [nc.sync, nc.scalar]  # the two HWDGE rings

    with tile.TileContext(nc) as tc:
        with ExitStack() as ctx:
            const = ctx.enter_context(tc.tile_pool(name="const", bufs=1))
            wpool = ctx.enter_context(tc.tile_pool(name="wpool", bufs=13))
            spool = ctx.enter_context(tc.tile_pool(name="spool", bufs=4))
            pspool = ctx.enter_context(
                tc.tile_pool(name="pspool", bufs=NT * BT + 1, space="PSUM")
            )

            svT = const.tile([128, KT, Bfull], bf16)

            # all PSUM accumulators live for the whole kernel (NT*BT banks)
            ps = {
                (m, n): pspool.tile([128, NTILES[n]], f32, name=f"ps{m}_{n}", tag="ps")
                for n in range(NT)
                for m in range(BT)
            }

            # junk-matmul machinery: K=1 N=128 matmuls with no DMA deps into
            # a spare 7th PSUM bank, bridging the preamble -> first-W gap.
            # K=1 junk is INVISIBLE to the HAM clock-gate (1/128 array rows
            # active) -- deliberately so.  Full-K junk that warms the PE at
            # ~11us was tried and measured WORSE: the front is supply-bound,
            # so a cold PE loses nothing (stall time and cold time trade
            # ~1:1) while building a ~2.3MB prefetch cushion by the time the
            # real matmul stream warms the clock (~20us); a warm PE instead
            # rides the DMA stream just-in-time with zero cushion, turning
            # every DMA hiccup into a PE stall and oscillating the HAM gate.
            psj = pspool.tile([128, 128], f32, name="psj", tag="ps")
            jw = const.tile([1, 128], bf16)
            nc.vector.memset(jw[:], 1.0)
            jr = const.tile([1, 128], bf16)
            nc.vector.memset(jr[:], 0.0)

            def junk(k):
                for _ in range(k):
                    nc.tensor.matmul(
                        psj[:], jw[:], jr[:],
                        start=True, stop=True, skip_group_check=True,
                    )

            junk(WARM)

            # greedy byte-balanced ring assignment: plain alternation with 4
            # DMAs/step never rotates parity and left one FIFO ring carrying
            # 17MB vs 9MB -- the heavy ring's blocks then arrive late and
            # stall the PE mid-kernel
            ring_bytes = [0, 0]

            def next_ring(nbytes):
                i = 0 if ring_bytes[0] <= ring_bytes[1] else 1
                ring_bytes[i] += nbytes
                return rings[i]

            def load_svt(k0, nkt):
                # svT rides the HWDGE rings like W (SWDGE was tried as a
                # third lane and is far too slow as an input path, ~55GB/s)
                next_ring(nkt * 512).dma_start(
                    svT[:, k0:k0 + nkt, :], svT_d[:, k0:k0 + nkt, :]
                )

            # diagonal wavefront: step t covers (g=t-n, n) for each tile n
            for t in range(NG + NT - 1):
                diag = [(t - n, n) for n in range(NT - 1, -1, -1) if 0 <= t - n < NG]
                # DMA emission in STRICT consumption order: W blocks oldest
                # diagonal first, and svT chunk g's pieces ride immediately
                # ahead of their consumer W(g, 0)'s pieces.  The ramp is
                # supply-bound, so any byte queued ahead of its need slot
                # directly delays the PE.
                wts = {}
                for g, n in diag:
                    ktl = KGRPS[g]
                    blk = W_ds[n][KOFF[g] * 128 * NTILES[n]:(KOFF[g] + ktl) * 128 * NTILES[n]]
                    blk = blk.rearrange("(p r) -> p r", p=128)
                    svt_splits = list(SVT_SPLITS.get(g, (ktl,))) if (n == 0 and g == 0) else []
                    svt_off = 0
                    parts, off = [], 0
                    for sk in W_SPLITS.get((g, n), (ktl,)):
                        # svT(0) pieces ride just ahead of the matching
                        # W(0,0) pieces (both on the opening critical path)
                        while svt_splits and svt_off < off + sk:
                            svk = svt_splits.pop(0)
                            load_svt(KOFF[g] + svt_off, svk)
                            svt_off += svk
                        wt = wpool.tile([128, sk, NTILES[n]], bf16, name="wt", tag="wt")
                        next_ring(sk * NTILES[n] * 2).dma_start(
                            wt[:], blk[:, off * NTILES[n]:(off + sk) * NTILES[n]]
                        )
                        parts.append((wt, sk))
                        off += sk
                    wts[(g, n)] = parts
                # svT chunk t+1 trails step t's W blocks: one step of lead
                # over its consumer (t+1, 0) -- strict zero-lead placement
                # left no slack and stalled the PE whenever DMA hiccupped
                if t + 1 < NG:
                    cum = 0
                    for svk in SVT_SPLITS.get(t + 1, (KGRPS[t + 1],)):
                        load_svt(KOFF[t + 1] + cum, svk)
                        cum += svk
                # matmuls: oldest diagonal first; m outer within each part so
                # within a tile's final group m=0's accumulation closes (and
                # drains) while m=1 still streams.  Ramp seams get junk pads.
                unit = 0
                for g, n in diag:
                    koff = KOFF[g]
                    for wt, nkt in wts[(g, n)]:
                        for m in range(BT):
                            for ktl in range(nkt):
                                kt = koff + ktl
                                nc.tensor.matmul(
                                    ps[(m, n)][:],
                                    svT[:, kt, m * 128:(m + 1) * 128],
                                    wt[:, ktl],
                                    start=(kt == 0),
                                    stop=(kt == KT - 1),
                                )
                        koff += nkt
                        junk(PADS.get((t, unit), 0))
                        unit += 1
                    assert koff == KOFF[g] + KGRPS[g]
                # drains for any tile whose last group just ran: one
                # PSUM->SBUF bf16 cast on DVE, then a single contiguous-line
                # output DMA.  Mid drains ride SWDGE (HWDGE rings are
                # mid-W-stream); the final tile's drains use the by-then-idle
                # HWDGE rings.
                for g, n in diag:
                    if g != NG - 1:
                        continue
                    for m in range(BT):
                        st = spool.tile([128, NTILES[n]], bf16, name="st", tag="st")
                        nc.vector.tensor_copy(st[:], ps[(m, n)][:])
                        base = m * OF * D1 + NOFF[n]
                        if n < NT - 1:
                            nc.gpsimd.dma_start(
                                out_d[:, base:base + NTILES[n]], st[:]
                            )
                        else:
                            # final tile: the serial tail -- split each drain
                            # across both (by now idle) HWDGE rings so issue,
                            # transfer, and HBM-write receipt run in parallel
                            h = NTILES[n] // 2
                            rings[0].dma_start(out_d[:, base:base + h], st[:, :h])
                            rings[1].dma_start(out_d[:, base + h:base + NTILES[n]], st[:, h:])

    nc.finalize()
    return nc


_PROGRAM = None


def _get_program():
    global _PROGRAM
    if _PROGRAM is None:
        _PROGRAM = build_core_program(**FULL_CFG)
    return _PROGRAM


def _prep_inputs(x, W, b):
    bf16 = ml_dtypes.bfloat16
    KT, NTILES = FULL_CFG["KT"], FULL_CFG["NTILES"]
    NOFF = [sum(NTILES[:i]) for i in range(len(NTILES))]
    # svT[p, kt, m] = sv[m, kt*128 + p], sv = x[:, :, SV_IDX] flattened
    sv = np.ascontiguousarray(x[:, :, SV_IDX]).reshape(B, IN_F * D1)
    svT = np.ascontiguousarray(sv.reshape(B, KT, 128).transpose(2, 1, 0)).astype(bf16)

    Wb = W.astype(bf16)
    # Wr[c, o', kt, p] with o' the core-local output column
    Wr = Wb.reshape(NCORES, OF * D1, KT, 128)
    KTLS = FULL_CFG["KTLS"]
    KOFFS = [[sum(k[:i]) for i in range(len(k))] for k in KTLS]
    in_maps = []
    for c in range(NCORES):
        m = {"svT": svT}
        for n, nt in enumerate(NTILES):
            # per k-group block [p, ktl, jj] = W_core[NOFF[n]+jj, kt*128+p],
            # raveled + concatenated (matches the device-side slices)
            sub = Wr[c, NOFF[n]:NOFF[n] + nt]  # [jj, kt, p]
            parts = []
            for g, ktl in enumerate(KTLS[n]):
                a = KOFFS[n][g]
                blk = sub[:, a:a + ktl]  # [jj, ktl, p]
                parts.append(np.ascontiguousarray(blk.transpose(2, 1, 0)).ravel())
            m[f"Wh{n}"] = np.concatenate(parts)
        in_maps.append(m)
    return in_maps


def run(x, W, b, trace=False):
    x = np.asarray(x, dtype=np.float32)
    W = np.asarray(W, dtype=np.float32)
    b = np.asarray(b, dtype=np.float32)
    in_maps = _prep_inputs(x, W, b)
    nc = _get_program()
    res = None
    for attempt in range(3):
        try:
            res = run_bass_kernel_spmd(
                nc, in_maps, core_ids=list(range(NCORES)), trace=trace
            )
            break
        except Exception:
            if attempt == 2:
                raise
            import time as _time
            _time.sleep(5)
    # host-side epilogue in f32: de-interleave [p, m, j] -> [m*128+p, j],
    # then bias, bivector products, scatter
    BT = FULL_CFG["BT"]
    svo = np.concatenate(
        [
            np.asarray(res.results[c]["outc"])
            .reshape(128, BT, N_CORE)
            .transpose(1, 0, 2)
            .reshape(B, N_CORE)
            for c in range(NCORES)
        ],
        axis=1,
    ).astype(np.float32)
    svo += b[None, :]
    svo = svo.reshape(B, OUT_F, D1)
    v = svo[:, :, 1:]
    biv = v[:, :, IU] * v[:, :, JU]
    out = np.zeros((B, OUT_F, MV_DIM), dtype=np.float32)
    out[:, :, SV_IDX] = svo
    out[:, :, BIV_IDX] = biv
    return out, res


def kernel(x, W, b):
    out, _ = run(x, W, b)
    return out


# revision 31
# speedup vs baseline: 1.0006x; 1.0006x over previous
"""CliffordLinearSimple on 8 Trainium2 NeuronCores.

Math (per reference):
    sv   = x[:, :, SV_IDX]                      # [B, IN_F, 9]  (scalar+vector slots)
    svo  = sv.reshape(B, IN_F*9) @ W.T + b      # [B, OUT_F*9]
    v    = svo.reshape(B, OUT_F, 9)[:, :, 1:]   # [B, OUT_F, 8]
    biv  = v[:, :, IU] * v[:, :, JU]            # [B, OUT_F, 28]
    out[..., SV_IDX] = svo; out[..., BIV_IDX] = biv; rest 0

Distribution: tensor-parallel over OUT_F (row-split W): core c owns out
features [c*128, (c+1)*128).  Every core gets the full sv (gathered and
transposed on host -- only 9/256 of x's last dim is ever read), and its
W row shard pre-packed to the PE's [K, N] layout in bf16.

The device does ONLY the GEMM (bf16 operands, fp32 PSUM) and writes the
[256, 1152] result back as bf16 (~0.6MB): bias add, the 28 bivector
products, and the scatter into the [256, 1024, 256] multivector output
happen on the host in fp32.

Schedule (v2, diagonal k-outer): the N=1152 columns are three PSUM
tiles (432, 504, 216); K=9216 is 72 k-tiles in groups
G=(6,9,9,12,12,12,12).  Step t runs W-blocks (g=t, n=0), (g=t-1, n=1),
(g=t-2, n=2) -- a diagonal wavefront.  This (a) streams W and svT at a
uniform byte rate matched to PE consumption with no phase cliffs (the
old n-outer schedule front-loaded svT + the first column tile and
stalled the PE ~4.4us at phase boundaries), (b) opens with a split
2-ktile W chunk so real matmuls begin ~1.5us after the framework
preamble, and (c) finishes the three tiles staggered (steps 6, 7, 8),
so mid drains overlap compute on SWDGE and only the narrow 216-wide
tile's drain is in the serial tail, split across the then-idle HWDGE
rings.  A handful of K=1 junk warm-up matmuls (HAM-invisible by
design; see the junk() comment) keep the PE queue alive between the
preamble and the first W chunk.
"""
import sys

if "/opt/trn_rl_repo" not in sys.path:
    sys.path.insert(0, "/opt/trn_rl_repo")

from contextlib import ExitStack

import ml_dtypes
import numpy as np

import concourse.bass as bass
import concourse.tile as tile
from concourse import bacc, mybir
from concourse.bass_utils import run_bass_kernel_spmd

ALG_DIM = 8
D1 = 9
MV_DIM = 256
B, IN_F, OUT_F = 256, 1024, 1024
POW2 = np.array([2 ** i for i in range(ALG_DIM)])
SV_IDX = np.concatenate([[0], POW2])
IU, JU = np.triu_indices(ALG_DIM, 1)
BIV_IDX = POW2[IU] + POW2[JU]
NCORES = 8
OF = OUT_F // NCORES  # 128 out features per core
N_CORE = OF * D1      # 1152 out slots per core

# K = IN_F*9 = 9216 = KT*128.  KGRP: k-group sizes (in 128-deep k-tiles)
# shared by all three column tiles; NTILES: PSUM tile widths (<=512
# f32/bank), narrowest last so the final drain is smallest.  Groups are
# big (>=6kt) so each W block carries >=2us of PE work -- with small
# ramped groups the ~0.65us/DMA ring-issue latency plus transfer time
# couldn't hide inside the tiny early steps and the PE chased the DMA
# stream just-in-time-late for the whole first half (measured +13.6us
# of ramp stalls).  Only the very first (g=0, n=0) block is split 2+4
# ktiles so the opening matmuls start ~1us earlier.
KGRP = (6, 9, 9, 12, 12, 12, 12)
FULL_CFG = dict(KT=72, KTLS=(KGRP, KGRP, KGRP), OF=128, NTILES=(432, 504, 216), BT=2, WARM=20)

# ramp-phase DMA splits (in ktiles) for W block (0,0) and svT chunk 0 --
# the opening critical path only.  Finer splits were tried and HURT:
# sub-2KB partition lines crater SDMA descriptor efficiency and each
# piece pays a ~0.7us fixed floor.
W_SPLITS = {(0, 0): (2, 4)}
SVT_SPLITS = {0: (2, 4)}

# junk-matmul padding (count of junk MMs) emitted after the i-th
# (0-based) ramp seam, keyed by (step t, index in that step's MM
# emission).  Empty: static pads execute when the PE reaches them, not
# when a stall actually happens, so they cost their full duration on a
# fast device -- measured as pure overhead (+3.6us).
PADS = {}


def build_core_program(KT, KTLS, OF, NTILES, BT, WARM=0):
    """SPMD per-core program: C[128*BT, OF*9] = svT.T @ Wh, written back as
    bf16 (bias + bivector products happen on the host)."""
    assert all(KT == sum(k) for k in KTLS) and sum(NTILES) == OF * D1
    NT = len(NTILES)
    assert len(KTLS) == NT
    KGRPS = KTLS[0]
    assert all(tuple(k) == tuple(KGRPS) for k in KTLS)
    NG = len(KGRPS)
    NOFF = [sum(NTILES[:i]) for i in range(NT)]          # column offsets
    KOFF = [sum(KGRPS[:i]) for i in range(NG)]           # k-group offsets
    Bfull = BT * 128
    f32, bf16 = mybir.dt.float32, mybir.dt.bfloat16

    nc = bacc.Bacc("TRN2", target_bir_lowering=False, debug=False)
    svT_d = nc.dram_tensor("svT", [128, KT, Bfull], bf16, kind="ExternalInput").ap()
    # flat per-n W: k-group blocks [128, ktl, NTILE] packed contiguously in
    # group order, so every DMA reads one fully-sequential DRAM region
    W_ds = [
        nc.dram_tensor(f"Wh{n}", [KT * 128 * NTILES[n]], bf16, kind="ExternalInput").ap()
        for n in range(NT)
    ]
    # [p, m*1152 + j] = C[m*128 + p, j]: per-partition output lines are
    # contiguous, so each drain is one 128-descriptor DMA
    out_d = nc.dram_tensor("outc", [128, BT * OF * D1], bf16, kind="ExternalOutput").ap()

    rings = [nc.sync, nc.scalar]  # the two HWDGE rings

    with tile.TileContext(nc) as tc:
        with ExitStack() as ctx:
            const = ctx.enter_context(tc.tile_pool(name="const", bufs=1))
            wpool = ctx.enter_context(tc.tile_pool(name="wpool", bufs=13))
            spool = ctx.enter_context(tc.tile_pool(name="spool", bufs=4))
            pspool = ctx.enter_context(
                tc.tile_pool(name="pspool", bufs=NT * BT + 1, space="PSUM")
            )

            svT = const.tile([128, KT, Bfull], bf16)

            # all PSUM accumulators live for the whole kernel (NT*BT banks)
            ps = {
                (m, n): pspool.tile([128, NTILES[n]], f32, name=f"ps{m}_{n}", tag="ps")
                for n in range(NT)
                for m in range(BT)
            }

            # junk-matmul machinery: K=1 N=128 matmuls with no DMA deps into
            # a spare 7th PSUM bank, bridging the preamble -> first-W gap.
            # K=1 junk is INVISIBLE to the HAM clock-gate (1/128 array rows
            # active) -- deliberately so.  Full-K junk that warms the PE at
            # ~11us was tried and measured WORSE: the front is supply-bound,
            # so a cold PE loses nothing (stall time and cold time trade
            # ~1:1) while building a ~2.3MB prefetch cushion by the time the
            # real matmul stream warms the clock (~20us); a warm PE instead
            # rides the DMA stream just-in-time with zero cushion, turning
            # every DMA hiccup into a PE stall and oscillating the HAM gate.
            psj = pspool.tile([128, 128], f32, name="psj", tag="ps")
            jw = const.tile([1, 128], bf16)
            nc.vector.memset(jw[:], 1.0)
            jr = const.tile([1, 128], bf16)
            nc.vector.memset(jr[:], 0.0)

            def junk(k):
                for _ in range(k):
                    nc.tensor.matmul(
                        psj[:], jw[:], jr[:],
                        start=True, stop=True, skip_group_check=True,
                    )

            junk(WARM)

            # greedy byte-balanced ring assignment: plain alternation with 4
            # DMAs/step never rotates parity and left one FIFO ring carrying
            # 17MB vs 9MB -- the heavy ring's blocks then arrive late and
            # stall the PE mid-kernel
            ring_bytes = [0, 0]

            def next_ring(nbytes):
                i = 0 if ring_bytes[0] <= ring_bytes[1] else 1
                ring_bytes[i] += nbytes
                return rings[i]

            def load_svt(k0, nkt):
                # svT rides the HWDGE rings like W (SWDGE was tried as a
                # third lane and is far too slow as an input path, ~55GB/s)
                next_ring(nkt * 512).dma_start(
                    svT[:, k0:k0 + nkt, :], svT_d[:, k0:k0 + nkt, :]
                )

            # diagonal wavefront: step t covers (g=t-n, n) for each tile n
            for t in range(NG + NT - 1):
                diag = [(t - n, n) for n in range(NT - 1, -1, -1) if 0 <= t - n < NG]
                # DMA emission in STRICT consumption order: W blocks oldest
                # diagonal first, and svT chunk g's pieces ride immediately
                # ahead of their consumer W(g, 0)'s pieces.  The ramp is
                # supply-bound, so any byte queued ahead of its need slot
                # directly delays the PE.
                wts = {}
                for g, n in diag:
                    ktl = KGRPS[g]
                    blk = W_ds[n][KOFF[g] * 128 * NTILES[n]:(KOFF[g] + ktl) * 128 * NTILES[n]]
                    blk = blk.rearrange("(p r) -> p r", p=128)
                    svt_splits = list(SVT_SPLITS.get(g, (ktl,))) if (n == 0 and g == 0) else []
                    svt_off = 0
                    parts, off = [], 0
                    for sk in W_SPLITS.get((g, n), (ktl,)):
                        # svT(0) pieces ride just ahead of the matching
                        # W(0,0) pieces (both on the opening critical path)
                        while svt_splits and svt_off < off + sk:
                            svk = svt_splits.pop(0)
                            load_svt(KOFF[g] + svt_off, svk)
                            svt_off += svk
                        wt = wpool.tile([128, sk, NTILES[n]], bf16, name="wt", tag="wt")
                        next_ring(sk * NTILES[n] * 2).dma_start(
                            wt[:], blk[:, off * NTILES[n]:(off + sk) * NTILES[n]]
                        )
                        parts.append((wt, sk))
                        off += sk
                    wts[(g, n)] = parts
                # svT chunk t+1 trails step t's W blocks: one step of lead
                # over its consumer (t+1, 0) -- strict zero-lead placement
                # left no slack and stalled the PE whenever DMA hiccupped
                if t + 1 < NG:
                    cum = 0
                    for svk in SVT_SPLITS.get(t + 1, (KGRPS[t + 1],)):
                        load_svt(KOFF[t + 1] + cum, svk)
                        cum += svk
                # matmuls: oldest diagonal first; m outer within each part so
                # within a tile's final group m=0's accumulation closes (and
                # drains) while m=1 still streams.  Ramp seams get junk pads.
                unit = 0
                for g, n in diag:
                    koff = KOFF[g]
                    for wt, nkt in wts[(g, n)]:
                        for m in range(BT):
                            for ktl in range(nkt):
                                kt = koff + ktl
                                nc.tensor.matmul(
                                    ps[(m, n)][:],
                                    svT[:, kt, m * 128:(m + 1) * 128],
                                    wt[:, ktl],
                                    start=(kt == 0),
                                    stop=(kt == KT - 1),
                                )
                        koff += nkt
                        junk(PADS.get((t, unit), 0))
                        unit += 1
                    assert koff == KOFF[g] + KGRPS[g]
                # drains for any tile whose last group just ran: one
                # PSUM->SBUF bf16 cast on DVE, then a single contiguous-line
                # output DMA.  Mid drains ride SWDGE (HWDGE rings are
                # mid-W-stream); the final tile's drains use the by-then-idle
                # HWDGE rings.
                for g, n in diag:
                    if g != NG - 1:
                        continue
                    for m in range(BT):
                        st = spool.tile([128, NTILES[n]], bf16, name="st", tag="st")
                        nc.vector.tensor_copy(st[:], ps[(m, n)][:])
                        base = m * OF * D1 + NOFF[n]
                        if n < NT - 1:
                            nc.gpsimd.dma_start(
                                out_d[:, base:base + NTILES[n]], st[:]
                            )
                        else:
                            # final tile: the serial tail -- split each drain
                            # across both (by now idle) HWDGE rings so issue,
                            # transfer, and HBM-write receipt run in parallel
                            h = NTILES[n] // 2
                            rings[0].dma_start(out_d[:, base:base + h], st[:, :h])
                            rings[1].dma_start(out_d[:, base + h:base + NTILES[n]], st[:, h:])

    nc.finalize()
    return nc


_PROGRAM = None


def _get_program():
    global _PROGRAM
    if _PROGRAM is None:
        _PROGRAM = build_core_program(**FULL_CFG)
    return _PROGRAM


def _prep_inputs(x, W, b):
    bf16 = ml_dtypes.bfloat16
    KT, NTILES = FULL_CFG["KT"], FULL_CFG["NTILES"]
    NOFF = [sum(NTILES[:i]) for i in range(len(NTILES))]
    # svT[p, kt, m] = sv[m, kt*128 + p], sv = x[:, :, SV_IDX] flattened
    sv = np.ascontiguousarray(x[:, :, SV_IDX]).reshape(B, IN_F * D1)
    svT = np.ascontiguousarray(sv.reshape(B, KT, 128).transpose(2, 1, 0)).astype(bf16)

    Wb = W.astype(bf16)
    # Wr[c, o', kt, p] with o' the core-local output column
    Wr = Wb.reshape(NCORES, OF * D1, KT, 128)
    KTLS = FULL_CFG["KTLS"]
    KOFFS = [[sum(k[:i]) for i in range(len(k))] for k in KTLS]
    in_maps = []
    for c in range(NCORES):
        m = {"svT": svT}
        for n, nt in enumerate(NTILES):
            # per k-group block [p, ktl, jj] = W_core[NOFF[n]+jj, kt*128+p],
            # raveled + concatenated (matches the device-side slices)
            sub = Wr[c, NOFF[n]:NOFF[n] + nt]  # [jj, kt, p]
            parts = []
            for g, ktl in enumerate(KTLS[n]):
                a = KOFFS[n][g]
                blk = sub[:, a:a + ktl]  # [jj, ktl, p]
                parts.append(np.ascontiguousarray(blk.transpose(2, 1, 0)).ravel())
            m[f"Wh{n}"] = np.concatenate(parts)
        in_maps.append(m)
    return in_maps


def run(x, W, b, trace=False):
    x = np.asarray(x, dtype=np.float32)
    W = np.asarray(W, dtype=np.float32)
    b = np.asarray(b, dtype=np.float32)
    in_maps = _prep_inputs(x, W, b)
    nc = _get_program()
    res = None
    for attempt in range(3):
        try:
            res = run_bass_kernel_spmd(
                nc, in_maps, core_ids=list(range(NCORES)), trace=trace
            )
            break
        except Exception:
            if attempt == 2:
                raise
            import time as _time
            _time.sleep(5)
    # host-side epilogue in f32: de-interleave [p, m, j] -> [m*128+p, j],
    # then bias, bivector products, scatter
    BT = FULL_CFG["BT"]
    svo = np.concatenate(
        [
            np.asarray(res.results[c]["outc"])
            .reshape(128, BT, N_CORE)
            .transpose(1, 0, 2)
            .reshape(B, N_CORE)
            for c in range(NCORES)
        ],
        axis=1,
    ).astype(np.float32)
    svo += b[None, :]
    svo = svo.reshape(B, OUT_F, D1)
    v = svo[:, :, 1:]
    biv = v[:, :, IU] * v[:, :, JU]
    out = np.zeros((B, OUT_F, MV_DIM), dtype=np.float32)
    out[:, :, SV_IDX] = svo
    out[:, :, BIV_IDX] = biv
    return out, res


def kernel(x, W, b):
    out, _ = run(x, W, b)
    return out


# revision 33
# speedup vs baseline: 1.0686x; 1.0679x over previous
"""CliffordLinearSimple on 8 Trainium2 NeuronCores.

Math (per reference):
    sv   = x[:, :, SV_IDX]                      # [B, IN_F, 9]  (scalar+vector slots)
    svo  = sv.reshape(B, IN_F*9) @ W.T + b      # [B, OUT_F*9]
    v    = svo.reshape(B, OUT_F, 9)[:, :, 1:]   # [B, OUT_F, 8]
    biv  = v[:, :, IU] * v[:, :, JU]            # [B, OUT_F, 28]
    out[..., SV_IDX] = svo; out[..., BIV_IDX] = biv; rest 0

Distribution: tensor-parallel over OUT_F (row-split W): core c owns out
features [c*128, (c+1)*128).  Every core gets the full sv (gathered and
transposed on host -- only 9/256 of x's last dim is ever read), and its
W row shard pre-packed to the PE's [K, N] layout in bf16.

The device does ONLY the GEMM (bf16 operands, fp32 PSUM) and writes the
[256, 1152] result back as bf16 (~0.6MB): bias add, the 28 bivector
products, and the scatter into the [256, 1024, 256] multivector output
happen on the host in fp32.

Schedule (v2, diagonal k-outer): the N=1152 columns are three PSUM
tiles (432, 504, 216); K=9216 is 72 k-tiles in groups
G=(6,9,9,12,12,12,12).  Step t runs W-blocks (g=t, n=0), (g=t-1, n=1),
(g=t-2, n=2) -- a diagonal wavefront.  This (a) streams W and svT at a
uniform byte rate matched to PE consumption with no phase cliffs (the
old n-outer schedule front-loaded svT + the first column tile and
stalled the PE ~4.4us at phase boundaries), (b) opens with a split
2-ktile W chunk so real matmuls begin ~1.5us after the framework
preamble, and (c) finishes the three tiles staggered (steps 6, 7, 8),
so mid drains overlap compute on SWDGE and only the narrow 216-wide
tile's drain is in the serial tail, split across the then-idle HWDGE
rings.  A handful of K=1 junk warm-up matmuls (HAM-invisible by
design; see the junk() comment) keep the PE queue alive between the
preamble and the first W chunk.
"""
import sys

if "/opt/trn_rl_repo" not in sys.path:
    sys.path.insert(0, "/opt/trn_rl_repo")

from contextlib import ExitStack

import ml_dtypes
import numpy as np

import concourse.bass as bass
import concourse.tile as tile
from concourse import bacc, mybir
from concourse.bass_utils import run_bass_kernel_spmd

ALG_DIM = 8
D1 = 9
MV_DIM = 256
B, IN_F, OUT_F = 256, 1024, 1024
POW2 = np.array([2 ** i for i in range(ALG_DIM)])
SV_IDX = np.concatenate([[0], POW2])
IU, JU = np.triu_indices(ALG_DIM, 1)
BIV_IDX = POW2[IU] + POW2[JU]
NCORES = 8
OF = OUT_F // NCORES  # 128 out features per core
N_CORE = OF * D1      # 1152 out slots per core

# K = IN_F*9 = 9216 = KT*128.  KGRP: k-group sizes (in 128-deep k-tiles)
# shared by all three column tiles; NTILES: PSUM tile widths (<=512
# f32/bank), narrowest last so the final drain is smallest.  Groups are
# big (>=6kt) so each W block carries >=2us of PE work -- with small
# ramped groups the ~0.65us/DMA ring-issue latency plus transfer time
# couldn't hide inside the tiny early steps and the PE chased the DMA
# stream just-in-time-late for the whole first half (measured +13.6us
# of ramp stalls).  Only the very first (g=0, n=0) block is split 2+4
# ktiles so the opening matmuls start ~1us earlier.
KGRP = (6, 9, 9, 12, 12, 12, 12)
FULL_CFG = dict(KT=72, KTLS=(KGRP, KGRP, KGRP), OF=128, NTILES=(432, 504, 216), BT=2, WARM=20)

# DMA splits (in ktiles) per W block (g, n) and svT chunk g.  The
# opening block (0,0) splits 2+4 for first-matmul latency.  Mid-kernel
# 12kt/9kt blocks split in half: a whole 12kt block is ~1.5MB = ~4.3us
# of supply, so when the device's DMA rate dips below PE demand the PE
# stalls in >3.4us lumps at block boundaries -- crossing the HAM MID
# window and re-throttling the clock to 1.2GHz (~1.7us extra per hit).
# Half blocks keep stall lumps under the threshold while per-partition
# lines stay >=5KB (full descriptor efficiency; sub-2KB lines measured
# ~60% slower -- do NOT split finer).
W_SPLITS = {(0, 0): (2, 4)}
SVT_SPLITS = {0: (2, 4)}
for _g, _k in enumerate(KGRP):
    _sp = (6, 6) if _k == 12 else ((5, 4) if _k == 9 else None)
    if _sp is None:
        continue
    SVT_SPLITS[_g] = _sp
    for _n in range(3):
        W_SPLITS[(_g, _n)] = _sp

# junk-matmul padding (count of junk MMs) emitted after the i-th
# (0-based) ramp seam, keyed by (step t, index in that step's MM
# emission).  Empty: static pads execute when the PE reaches them, not
# when a stall actually happens, so they cost their full duration on a
# fast device -- measured as pure overhead (+3.6us).
PADS = {}


def build_core_program(KT, KTLS, OF, NTILES, BT, WARM=0):
    """SPMD per-core program: C[128*BT, OF*9] = svT.T @ Wh, written back as
    bf16 (bias + bivector products happen on the host)."""
    assert all(KT == sum(k) for k in KTLS) and sum(NTILES) == OF * D1
    NT = len(NTILES)
    assert len(KTLS) == NT
    KGRPS = KTLS[0]
    assert all(tuple(k) == tuple(KGRPS) for k in KTLS)
    NG = len(KGRPS)
    NOFF = [sum(NTILES[:i]) for i in range(NT)]          # column offsets
    KOFF = [sum(KGRPS[:i]) for i in range(NG)]           # k-group offsets
    Bfull = BT * 128
    f32, bf16 = mybir.dt.float32, mybir.dt.bfloat16

    nc = bacc.Bacc("TRN2", target_bir_lowering=False, debug=False)
    svT_d = nc.dram_tensor("svT", [128, KT, Bfull], bf16, kind="ExternalInput").ap()
    # flat per-n W: k-group blocks [128, ktl, NTILE] packed contiguously in
    # group order, so every DMA reads one fully-sequential DRAM region
    W_ds = [
        nc.dram_tensor(f"Wh{n}", [KT * 128 * NTILES[n]], bf16, kind="ExternalInput").ap()
        for n in range(NT)
    ]
    # [p, m*1152 + j] = C[m*128 + p, j]: per-partition output lines are
    # contiguous, so each drain is one 128-descriptor DMA
    out_d = nc.dram_tensor("outc", [128, BT * OF * D1], bf16, kind="ExternalOutput").ap()

    rings = [nc.sync, nc.scalar]  # the two HWDGE rings

    with tile.TileContext(nc) as tc:
        with ExitStack() as ctx:
            const = ctx.enter_context(tc.tile_pool(name="const", bufs=1))
            wpool = ctx.enter_context(tc.tile_pool(name="wpool", bufs=20))
            spool = ctx.enter_context(tc.tile_pool(name="spool", bufs=4))
            pspool = ctx.enter_context(
                tc.tile_pool(name="pspool", bufs=NT * BT + 1, space="PSUM")
            )

            svT = const.tile([128, KT, Bfull], bf16)

            # all PSUM accumulators live for the whole kernel (NT*BT banks)
            ps = {
                (m, n): pspool.tile([128, NTILES[n]], f32, name=f"ps{m}_{n}", tag="ps")
                for n in range(NT)
                for m in range(BT)
            }

            # junk-matmul machinery: K=1 N=128 matmuls with no DMA deps into
            # a spare 7th PSUM bank, bridging the preamble -> first-W gap.
            # K=1 junk is INVISIBLE to the HAM clock-gate (1/128 array rows
            # active) -- deliberately so.  Full-K junk that warms the PE at
            # ~11us was tried and measured WORSE: the front is supply-bound,
            # so a cold PE loses nothing (stall time and cold time trade
            # ~1:1) while building a ~2.3MB prefetch cushion by the time the
            # real matmul stream warms the clock (~20us); a warm PE instead
            # rides the DMA stream just-in-time with zero cushion, turning
            # every DMA hiccup into a PE stall and oscillating the HAM gate.
            psj = pspool.tile([128, 128], f32, name="psj", tag="ps")
            jw = const.tile([1, 128], bf16)
            nc.vector.memset(jw[:], 1.0)
            jr = const.tile([1, 128], bf16)
            nc.vector.memset(jr[:], 0.0)

            def junk(k):
                for _ in range(k):
                    nc.tensor.matmul(
                        psj[:], jw[:], jr[:],
                        start=True, stop=True, skip_group_check=True,
                    )

            junk(WARM)

            # greedy byte-balanced ring assignment: plain alternation with 4
            # DMAs/step never rotates parity and left one FIFO ring carrying
            # 17MB vs 9MB -- the heavy ring's blocks then arrive late and
            # stall the PE mid-kernel
            ring_bytes = [0, 0]

            def next_ring(nbytes):
                i = 0 if ring_bytes[0] <= ring_bytes[1] else 1
                ring_bytes[i] += nbytes
                return rings[i]

            def load_svt(k0, nkt):
                # svT rides the HWDGE rings like W (SWDGE was tried as a
                # third lane and is far too slow as an input path, ~55GB/s)
                next_ring(nkt * 512).dma_start(
                    svT[:, k0:k0 + nkt, :], svT_d[:, k0:k0 + nkt, :]
                )

            # diagonal wavefront: step t covers (g=t-n, n) for each tile n
            for t in range(NG + NT - 1):
                diag = [(t - n, n) for n in range(NT - 1, -1, -1) if 0 <= t - n < NG]
                # DMA emission in STRICT consumption order: W blocks oldest
                # diagonal first, and svT chunk g's pieces ride immediately
                # ahead of their consumer W(g, 0)'s pieces.  The ramp is
                # supply-bound, so any byte queued ahead of its need slot
                # directly delays the PE.
                wts = {}
                for g, n in diag:
                    ktl = KGRPS[g]
                    blk = W_ds[n][KOFF[g] * 128 * NTILES[n]:(KOFF[g] + ktl) * 128 * NTILES[n]]
                    blk = blk.rearrange("(p r) -> p r", p=128)
                    svt_splits = list(SVT_SPLITS.get(g, (ktl,))) if (n == 0 and g == 0) else []
                    svt_off = 0
                    parts, off = [], 0
                    for sk in W_SPLITS.get((g, n), (ktl,)):
                        # svT(0) pieces ride just ahead of the matching
                        # W(0,0) pieces (both on the opening critical path)
                        while svt_splits and svt_off < off + sk:
                            svk = svt_splits.pop(0)
                            load_svt(KOFF[g] + svt_off, svk)
                            svt_off += svk
                        wt = wpool.tile([128, sk, NTILES[n]], bf16, name="wt", tag="wt")
                        next_ring(sk * NTILES[n] * 2).dma_start(
                            wt[:], blk[:, off * NTILES[n]:(off + sk) * NTILES[n]]
                        )
                        parts.append((wt, sk))
                        off += sk
                    wts[(g, n)] = parts
                # svT chunk t+1 trails step t's W blocks: one step of lead
                # over its consumer (t+1, 0) -- strict zero-lead placement
                # left no slack and stalled the PE whenever DMA hiccupped
                if t + 1 < NG:
                    cum = 0
                    for svk in SVT_SPLITS.get(t + 1, (KGRPS[t + 1],)):
                        load_svt(KOFF[t + 1] + cum, svk)
                        cum += svk
                # matmuls: oldest diagonal first; m outer within each part so
                # within a tile's final group m=0's accumulation closes (and
                # drains) while m=1 still streams.  Ramp seams get junk pads.
                unit = 0
                for g, n in diag:
                    koff = KOFF[g]
                    for wt, nkt in wts[(g, n)]:
                        for m in range(BT):
                            for ktl in range(nkt):
                                kt = koff + ktl
                                nc.tensor.matmul(
                                    ps[(m, n)][:],
                                    svT[:, kt, m * 128:(m + 1) * 128],
                                    wt[:, ktl],
                                    start=(kt == 0),
                                    stop=(kt == KT - 1),
                                )
                        koff += nkt
                        junk(PADS.get((t, unit), 0))
                        unit += 1
                    assert koff == KOFF[g] + KGRPS[g]
                # drains for any tile whose last group just ran: one
                # PSUM->SBUF bf16 cast on DVE, then a single contiguous-line
                # output DMA.  Mid drains ride SWDGE (HWDGE rings are
                # mid-W-stream); the final tile's drains use the by-then-idle
                # HWDGE rings.
                for g, n in diag:
                    if g != NG - 1:
                        continue
                    for m in range(BT):
                        st = spool.tile([128, NTILES[n]], bf16, name="st", tag="st")
                        nc.vector.tensor_copy(st[:], ps[(m, n)][:])
                        base = m * OF * D1 + NOFF[n]
                        if n < NT - 1:
                            nc.gpsimd.dma_start(
                                out_d[:, base:base + NTILES[n]], st[:]
                            )
                        else:
                            # final tile: the serial tail -- split each drain
                            # across both (by now idle) HWDGE rings so issue,
                            # transfer, and HBM-write receipt run in parallel
                            h = NTILES[n] // 2
                            rings[0].dma_start(out_d[:, base:base + h], st[:, :h])
                            rings[1].dma_start(out_d[:, base + h:base + NTILES[n]], st[:, h:])

    nc.finalize()
    return nc


_PROGRAM = None


def _get_program():
    global _PROGRAM
    if _PROGRAM is None:
        _PROGRAM = build_core_program(**FULL_CFG)
    return _PROGRAM


def _prep_inputs(x, W, b):
    bf16 = ml_dtypes.bfloat16
    KT, NTILES = FULL_CFG["KT"], FULL_CFG["NTILES"]
    NOFF = [sum(NTILES[:i]) for i in range(len(NTILES))]
    # svT[p, kt, m] = sv[m, kt*128 + p], sv = x[:, :, SV_IDX] flattened
    sv = np.ascontiguousarray(x[:, :, SV_IDX]).reshape(B, IN_F * D1)
    svT = np.ascontiguousarray(sv.reshape(B, KT, 128).transpose(2, 1, 0)).astype(bf16)

    Wb = W.astype(bf16)
    # Wr[c, o', kt, p] with o' the core-local output column
    Wr = Wb.reshape(NCORES, OF * D1, KT, 128)
    KTLS = FULL_CFG["KTLS"]
    KOFFS = [[sum(k[:i]) for i in range(len(k))] for k in KTLS]
    in_maps = []
    for c in range(NCORES):
        m = {"svT": svT}
        for n, nt in enumerate(NTILES):
            # per k-group block [p, ktl, jj] = W_core[NOFF[n]+jj, kt*128+p],
            # raveled + concatenated (matches the device-side slices)
            sub = Wr[c, NOFF[n]:NOFF[n] + nt]  # [jj, kt, p]
            parts = []
            for g, ktl in enumerate(KTLS[n]):
                a = KOFFS[n][g]
                blk = sub[:, a:a + ktl]  # [jj, ktl, p]
                parts.append(np.ascontiguousarray(blk.transpose(2, 1, 0)).ravel())
            m[f"Wh{n}"] = np.concatenate(parts)
        in_maps.append(m)
    return in_maps


def run(x, W, b, trace=False):
    x = np.asarray(x, dtype=np.float32)
    W = np.asarray(W, dtype=np.float32)
    b = np.asarray(b, dtype=np.float32)
    in_maps = _prep_inputs(x, W, b)
    nc = _get_program()
    res = None
    for attempt in range(3):
        try:
            res = run_bass_kernel_spmd(
                nc, in_maps, core_ids=list(range(NCORES)), trace=trace
            )
            break
        except Exception:
            if attempt == 2:
                raise
            import time as _time
            _time.sleep(5)
    # host-side epilogue in f32: de-interleave [p, m, j] -> [m*128+p, j],
    # then bias, bivector products, scatter
    BT = FULL_CFG["BT"]
    svo = np.concatenate(
        [
            np.asarray(res.results[c]["outc"])
            .reshape(128, BT, N_CORE)
            .transpose(1, 0, 2)
            .reshape(B, N_CORE)
            for c in range(NCORES)
        ],
        axis=1,
    ).astype(np.float32)
    svo += b[None, :]
    svo = svo.reshape(B, OUT_F, D1)
    v = svo[:, :, 1:]
    biv = v[:, :, IU] * v[:, :, JU]
    out = np.zeros((B, OUT_F, MV_DIM), dtype=np.float32)
    out[:, :, SV_IDX] = svo
    out[:, :, BIV_IDX] = biv
    return out, res


def kernel(x, W, b):
    out, _ = run(x, W, b)
    return out
